# revision 2
# baseline (speedup 1.0000x reference)
"""Rule-30 1D cellular automaton on 8 Trainium2 NeuronCores.

Problem: input [16, 2048] f32 -> threshold at 0.5 -> 1024 iterations of
elementary CA rule 30 (with wrap) -> output full history [16, 1025, 2048] uint8.

Sharding: data-parallel over batch, 2 rows per core, no collectives.

Per-core design:
  - Layout: row r (r=0,1) lives in partitions [64r, 64r+64); partition 64r+q
    owns cells [32q, 32q+32) of that row, plus G ghost cells each side
    (FD = 32 + 2G bytes per step). Cell values are the fp8e4m3 encoding of
    1.0 (0x38) or 0.0, so the TensorEngine can read the state directly.
  - Rule 30 reduces to new = right ^ (center | left): two dependent uint8
    bitwise vector ops per step over shifted views (bitwise preserves the
    0x00/0x38 encoding exactly; values are tiny so the fp32-internal ALU
    round-trips them).
  - Ghosts erode 1 cell/step; every G steps the TensorEngine rebuilds them
    with two block-ring permutation matmuls (fp8) into PSUM and the DVE
    copies PSUM back into the ghost slots.
  - The whole history (1025 steps x FD bytes/partition) stays resident in
    SBUF. After the last step the DVE bit-packs the interiors (8 cells ->
    1 byte, little-endian within the byte) into a [P, 1025*4] buffer that
    is DMA'd out -- 16x less wire traffic than the raw byte-per-cell dump,
    which matters because the axon host<->device tunnel runs at ~30 MB/s.
  - Host side unpacks with np.unpackbits.

Dispatch: the per-call jax plumbing is cached in a module global -- the
sharded jit executable is built once, the permutation-matrix constants are
committed to the devices once, and each call donates the previous call's
device-resident output as the next call's output-init buffer (the kernel
overwrites every output byte, so the init contents are irrelevant). This
removes the per-call retrace and the 4.2MB zeros upload that
run_bass_kernel_spmd would redo on every invocation.
"""
import contextlib
from concurrent.futures import ThreadPoolExecutor
import numpy as np

P = 128          # SBUF partitions
W = 2048         # CA width
T = 1024         # iterations
B = 16           # batch rows
NCORES = 8
IE = 2 * W // P  # interior cells per partition (32)
NR = P // 2      # partitions per row (64)
NT = T + 1       # history entries
PB = IE // 8     # packed bytes per partition per step (4)
RULE_TABLE = np.array([0, 1, 1, 1, 1, 0, 0, 0], dtype=np.uint8)

_NC_CACHE = {}
_ENGINE = None


def _build(T_steps, G):
    import concourse.bass as bass
    import concourse.mybir as mybir

    FD = IE + 2 * G
    NTl = T_steps + 1
    alu = mybir.AluOpType
    nc = bass.Bass(target_bir_lowering=False)

    x = nc.dram_tensor("x", [2, W], mybir.dt.float32, kind="ExternalInput")
    pdown = nc.dram_tensor("pdown", [P, P], mybir.dt.float8e4, kind="ExternalInput")
    pup = nc.dram_tensor("pup", [P, P], mybir.dt.float8e4, kind="ExternalInput")
    y = nc.dram_tensor("y", [P, NTl * PB], mybir.dt.uint8, kind="ExternalOutput")

    n_ref = (T_steps + G - 1) // G            # refreshes at t = 0, G, 2G, ...
    ref_steps = [k * G for k in range(n_ref)]

    with contextlib.ExitStack() as es:
        hist = es.enter_context(nc.sbuf_tensor([P, NTl * FD], mybir.dt.uint8))
        u = es.enter_context(nc.sbuf_tensor([P, FD], mybir.dt.uint8))
        fin = es.enter_context(nc.sbuf_tensor([P, IE], mybir.dt.float32))
        pmat = es.enter_context(nc.sbuf_tensor([P, 2 * P], mybir.dt.float8e4))
        acc = es.enter_context(nc.sbuf_tensor([P, NTl * PB], mybir.dt.uint8))
        tmp = es.enter_context(nc.sbuf_tensor([P, NTl * PB], mybir.dt.uint8))
        psum = es.enter_context(nc.psum_tensor([P, 2 * G], mybir.dt.float32))
        in_sem = es.enter_context(nc.semaphore("in_sem"))
        v_sem = es.enter_context(nc.semaphore("v_sem"))      # pack complete
        pe_go = es.enter_context(nc.semaphore("pe_go"))      # vector -> PE refresh request
        pe_done = es.enter_context(nc.semaphore("pe_done"))  # PE -> vector
        out_sem = es.enter_context(nc.semaphore("out_sem"))
        blk = es.enter_context(nc.Block())

        hist8 = hist[:].bitcast(mybir.dt.float8e4)   # fp8 view (same bytes)

        def tile(t):
            return hist[:, t * FD:(t + 1) * FD]

        def tile8(t):
            return hist8[:, t * FD:(t + 1) * FD]

        @blk.sync
        def _(sync):
            # input rows -> fin[128, 32]: partition 64r+q <- row r cells [32q, 32q+32)
            xr = x[:].rearrange("r (q c) -> (r q) c", c=IE)
            sync.dma_start(fin[:], xr).then_inc(in_sem, 16)
            sync.dma_start(pmat[:, 0:P], pdown[:]).then_inc(in_sem, 16)
            sync.dma_start(pmat[:, P:2 * P], pup[:]).then_inc(in_sem, 16)
            sync.wait_ge(v_sem, 1)
            sync.dma_start(y[:], acc[:]).then_inc(out_sem, 16)
            sync.wait_ge(out_sem, 16)

        @blk.tensor
        def _(tensor):
            tensor.wait_ge(in_sem, 48)
            for k, t in enumerate(ref_steps):
                tensor.wait_ge(pe_go, k + 1)
                # left ghosts: P_down @ interior tail [IE, IE+G)
                nc.tensor.matmul(psum[:, 0:G], pmat[:, 0:P],
                                 tile8(t)[:, IE:IE + G])
                # right ghosts: P_up @ interior head [G, 2G)
                inst = nc.tensor.matmul(psum[:, G:2 * G], pmat[:, P:2 * P],
                                        tile8(t)[:, G:2 * G])
                inst.then_inc(pe_done, 1)

        @blk.vector
        def _(vector):
            # The per-step XOR writes cols [1, FD-1); cols 0 and FD-1 of every
            # tile are read by the next step's OR but always eroded away.
            # Zero them once so reads are defined (and CoreSim is happy).
            h3 = hist[:].rearrange("p (t f) -> p t f", f=FD)
            nc.vector.memset(h3[:, :, 0:1], 0)
            nc.vector.memset(h3[:, :, FD - 1:FD], 0)
            vector.wait_ge(in_sem, 48)
            inst = nc.vector.tensor_scalar(tile8(0)[:, G:G + IE], fin[:],
                                           0.5, None, alu.is_ge)
            inst.then_inc(pe_go, 1)   # tile 0 interior complete -> refresh k=0
            for t in range(T_steps):
                if t in ref_steps:
                    k = ref_steps.index(t)
                    vector.wait_ge(pe_done, k + 1)
                    # Two copies (left/right ghost segments). NOTE: merging
                    # them into one 2-segment strided copy from PSUM passes
                    # CoreSim but corrupts ghost bytes on real hardware --
                    # keep the simple per-segment copies.
                    nc.vector.tensor_copy(tile8(t)[:, 0:G], psum[:, 0:G])
                    nc.vector.tensor_copy(tile8(t)[:, G + IE:FD],
                                          psum[:, G:2 * G])
                s = tile(t)
                d = tile(t + 1)
                # NOTE: erosion-aware shrunken per-step bounds (ops covering
                # only the still-valid [i, FD-i) range) pass analysis but
                # corrupt data on real hardware from mid-window steps onward;
                # keep the fixed full-width ops, which are HW-verified exact.
                nc.vector.tensor_tensor(u[:, 0:FD - 1], s[:, 0:FD - 1], s[:, 1:FD],
                                        alu.bitwise_or)
                inst = nc.vector.tensor_tensor(d[:, 1:FD - 1], u[:, 0:FD - 2],
                                               s[:, 2:FD], alu.bitwise_xor)
                if (t + 1) in ref_steps:
                    inst.then_inc(pe_go, 1)

            # Bit-pack the interiors: byte j of tile t on partition p holds
            # cells [8j, 8j+8) of that partition's 32-cell interior, bit i =
            # cell 8j+i. Interior cells sit at tile cols [G, G+IE) with G a
            # multiple of 8, so col = 8*(G//8 + j) + i.
            hv = hist8.rearrange("p (t j i) -> p t j i", j=FD // 8, i=8)
            av = acc[:].rearrange("p (t j) -> p t j", j=PB)
            tv = tmp[:].rearrange("p (t j) -> p t j", j=PB)
            j0 = G // 8
            nc.vector.tensor_scalar(av, hv[:, :, j0:j0 + PB, 0],
                                    0.5, None, alu.is_ge)
            for i in range(1, 8):
                nc.vector.tensor_scalar(tv, hv[:, :, j0:j0 + PB, i],
                                        0.5, float(1 << i), alu.is_ge, alu.mult)
                inst = nc.vector.tensor_tensor(av, av, tv, alu.add)
            inst.then_inc(v_sem, 1)

    return nc


def _perm_mats():
    """Block-ring permutation matrices (ring within each row's 64 partitions)."""
    import concourse.mybir as mybir
    f8 = mybir.dt.np(mybir.dt.float8e4)
    md = np.zeros((P, P), dtype=np.float32)
    mu = np.zeros((P, P), dtype=np.float32)
    for r in range(2):
        base = r * NR
        q = np.arange(NR)
        md[base + (q - 1) % NR, base + q] = 1.0   # out[m] = in[prev(m)]
        mu[base + (q + 1) % NR, base + q] = 1.0   # out[m] = in[next(m)]
    return md.astype(f8), mu.astype(f8)


def _get_nc(T_steps=T, G=16):
    key = (T_steps, G)
    if key not in _NC_CACHE:
        _NC_CACHE[key] = _build(T_steps, G)
    return _NC_CACHE[key]


def _get_engine():
    """Build (once) the cached sharded-jit dispatch for the 8-core kernel."""
    global _ENGINE
    if _ENGINE is not None:
        return _ENGINE
    import jax
    import concourse.mybir as mybir
    from concourse import bass2jax
    from jax.sharding import Mesh, PartitionSpec, NamedSharding
    from jax.experimental.shard_map import shard_map

    nc = _get_nc()
    bass2jax.install_neuronx_cc_hook()

    partition_name = nc.partition_id_tensor.name if nc.partition_id_tensor else None
    in_names, out_names, out_avals = [], [], []
    for alloc in nc.m.functions[0].allocations:
        if not isinstance(alloc, mybir.MemoryLocationSet):
            continue
        if alloc.kind not in ("ExternalInput", "ExternalOutput"):
            continue
        name = alloc.memorylocations[0].name
        if alloc.kind == "ExternalInput":
            if name != partition_name:
                in_names.append(name)
        else:
            out_names.append(name)
            out_avals.append(jax.core.ShapedArray(
                tuple(alloc.tensor_shape), mybir.dt.np(alloc.dtype)))
    assert in_names == ["x", "pdown", "pup"] and out_names == ["y"], \
        (in_names, out_names)
    all_names = in_names + out_names
    if partition_name is not None:
        all_names = all_names + [partition_name]
    n_params = len(in_names)
    donate = tuple(range(n_params, n_params + len(out_names)))

    def _body(*args):
        operands = list(args)
        if partition_name is not None:
            operands.append(bass2jax.partition_id_tensor())
        outs = bass2jax._bass_exec_p.bind(
            *operands,
            out_avals=tuple(out_avals),
            in_names=tuple(all_names),
            out_names=tuple(out_names),
            lowering_input_output_aliases=(),
            sim_require_finite=True,
            sim_require_nnan=True,
            nc=nc,
        )
        return tuple(outs)

    devices = jax.devices()[:NCORES]
    mesh = Mesh(np.asarray(devices), ("core",))
    nspecs = n_params + len(out_names)
    fn = jax.jit(
        shard_map(_body, mesh=mesh,
                  in_specs=(PartitionSpec("core"),) * nspecs,
                  out_specs=(PartitionSpec("core"),) * len(out_names),
                  check_rep=False),
        donate_argnums=donate, keep_unused=True,
    )

    md, mu = _perm_mats()
    sh = NamedSharding(mesh, PartitionSpec("core"))
    pd = jax.device_put(np.concatenate([md] * NCORES, axis=0), sh)
    pu = jax.device_put(np.concatenate([mu] * NCORES, axis=0), sh)
    pd.block_until_ready(), pu.block_until_ready()

    _ENGINE = {"fn": fn, "pd": pd, "pu": pu, "prev": None, "pool":
               ThreadPoolExecutor(max_workers=NCORES)}
    return _ENGINE


def _run_fast(inp):
    """inp: [16, 2048] f32 -> packed history [NCORES, P, NT*PB] uint8."""
    eng = _get_engine()
    x = np.ascontiguousarray(inp, dtype=np.float32)
    yinit = eng["prev"]
    eng["prev"] = None
    if yinit is None:
        yinit = np.zeros((NCORES * P, NT * PB), np.uint8)
    outs = eng["fn"](x, eng["pd"], eng["pu"], yinit)
    yg = outs[0]
    bufs = [None] * NCORES
    def fetch(s):
        bufs[s.index[0].start // P] = np.asarray(s.data)
    list(eng["pool"].map(fetch, yg.addressable_shards))
    eng["prev"] = yg
    return np.stack(bufs)


def _run_spmd(inp):
    """Fallback: same kernel through run_bass_kernel_spmd (slower per call)."""
    from concourse.bass_utils import run_bass_kernel_spmd
    nc = _get_nc()
    md, mu = _perm_mats()
    in_maps = [{"x": np.ascontiguousarray(inp[2 * i:2 * i + 2], dtype=np.float32),
                "pdown": md, "pup": mu} for i in range(NCORES)]
    res = run_bass_kernel_spmd(nc, in_maps, core_ids=list(range(NCORES)))
    return np.stack([res.results[i]["y"] for i in range(NCORES)])


def _unpack(y_all):
    """[NCORES, P, NT*PB] packed -> [16, NT, 2048] uint8 (0/1)."""
    a = y_all.reshape(NCORES, 2, NR, NT, PB)          # core, row, q, t, j
    a = np.ascontiguousarray(a.transpose(0, 1, 3, 2, 4))  # core, row, t, q, j
    a = a.reshape(B, NT, NR * PB)
    return np.unpackbits(a, axis=-1, bitorder="little")


def run_ca(inp):
    """inp: [16, 2048] f32. Returns [16, T+1, 2048] uint8 via the fast path,
    falling back to run_bass_kernel_spmd dispatch on any failure."""
    global _ENGINE
    try:
        return _unpack(_run_fast(inp))
    except Exception:
        _ENGINE = None
        return _unpack(_run_spmd(inp))


def _ca_reference_np(inp, lookup, iters):
    s = (inp >= 0.5).astype(np.uint8)
    hist = [s]
    for _ in range(iters):
        pad = np.concatenate([s[:, -1:], s, s[:, :1]], axis=1)
        idx = pad[:, :-2].astype(np.int32) + 2 * pad[:, 1:-1] + 4 * pad[:, 2:]
        s = lookup[idx].astype(np.uint8)
        hist.append(s)
    return np.stack(hist, axis=1)


def kernel(**inputs):
    inp = np.asarray(inputs["input"], dtype=np.float32)
    lookup = np.asarray(inputs["lookup"], dtype=np.uint8)
    if inp.shape != (B, W) or not np.array_equal(lookup, RULE_TABLE):
        # generic (non-rule-30 / odd-shape) fallback
        return _ca_reference_np(inp, lookup, T)
    return run_ca(inp)


# revision 4
# speedup vs baseline: 1.0946x; 1.0946x over previous
"""Rule-30 1D cellular automaton on 8 Trainium2 NeuronCores.

Problem: input [16, 2048] f32 -> threshold at 0.5 -> 1024 iterations of
elementary CA rule 30 (with wrap) -> output full history [16, 1025, 2048] uint8.

Sharding: data-parallel over batch, 2 rows per core, no collectives.

Per-core design:
  - Layout: row r (r=0,1) lives in partitions [64r, 64r+64); partition 64r+q
    owns cells [32q, 32q+32) of that row, plus G ghost cells each side
    (FD = 32 + 2G bytes per step). Cell values are the fp8e4m3 encoding of
    1.0 (0x38) or 0.0, so the TensorEngine can read the state directly.
  - Rule 30 reduces to new = right ^ (center | left): two dependent uint8
    bitwise vector ops per step over shifted views (bitwise preserves the
    0x00/0x38 encoding exactly; values are tiny so the fp32-internal ALU
    round-trips them).
  - Ghosts erode 1 cell/step; every G steps the TensorEngine rebuilds them
    with two block-ring permutation matmuls (fp8) into PSUM and the DVE
    copies PSUM back into the ghost slots.
  - The whole history (1025 steps x FD bytes/partition) stays resident in
    SBUF. After the last step the DVE bit-packs the interiors (8 cells ->
    1 byte, little-endian within the byte) into a [P, 1025*4] buffer that
    is DMA'd out -- 16x less wire traffic than the raw byte-per-cell dump,
    which matters because the axon host<->device tunnel runs at ~30 MB/s.
  - Host side unpacks with np.unpackbits.

Dispatch: the per-call jax plumbing is cached in a module global -- the
sharded jit executable is built once, the permutation-matrix constants are
committed to the devices once, and each call donates the previous call's
device-resident output as the next call's output-init buffer (the kernel
overwrites every output byte, so the init contents are irrelevant). This
removes the per-call retrace and the 4.2MB zeros upload that
run_bass_kernel_spmd would redo on every invocation.
"""
import contextlib
from concurrent.futures import ThreadPoolExecutor
import numpy as np

P = 128          # SBUF partitions
W = 2048         # CA width
T = 1024         # iterations
B = 16           # batch rows
NCORES = 8
IE = 2 * W // P  # interior cells per partition (32)
NR = P // 2      # partitions per row (64)
NT = T + 1       # history entries
PB = IE // 8     # packed bytes per partition per step (4)
RULE_TABLE = np.array([0, 1, 1, 1, 1, 0, 0, 0], dtype=np.uint8)

_NC_CACHE = {}
_ENGINE = None


def _build(T_steps, G):
    import concourse.bass as bass
    import concourse.mybir as mybir

    FD = IE + 2 * G
    NTl = T_steps + 1
    alu = mybir.AluOpType
    nc = bass.Bass(target_bir_lowering=False)

    x = nc.dram_tensor("x", [2, W], mybir.dt.float32, kind="ExternalInput")
    pdown = nc.dram_tensor("pdown", [P, P], mybir.dt.float8e4, kind="ExternalInput")
    pup = nc.dram_tensor("pup", [P, P], mybir.dt.float8e4, kind="ExternalInput")
    y = nc.dram_tensor("y", [P, NTl * PB], mybir.dt.uint8, kind="ExternalOutput")

    n_ref = (T_steps + G - 1) // G            # refreshes at t = 0, G, 2G, ...
    ref_steps = [k * G for k in range(n_ref)]

    with contextlib.ExitStack() as es:
        hist = es.enter_context(nc.sbuf_tensor([P, NTl * FD], mybir.dt.uint8))
        u = es.enter_context(nc.sbuf_tensor([P, FD], mybir.dt.uint8))
        fin = es.enter_context(nc.sbuf_tensor([P, IE], mybir.dt.float32))
        pmat = es.enter_context(nc.sbuf_tensor([P, 2 * P], mybir.dt.float8e4))
        acc = es.enter_context(nc.sbuf_tensor([P, NTl * PB], mybir.dt.uint8))
        tmp = es.enter_context(nc.sbuf_tensor([P, NTl * PB], mybir.dt.uint8))
        psum = es.enter_context(nc.psum_tensor([P, 2 * G], mybir.dt.float32))
        in_sem = es.enter_context(nc.semaphore("in_sem"))
        v_sem = es.enter_context(nc.semaphore("v_sem"))      # pack complete
        pe_go = es.enter_context(nc.semaphore("pe_go"))      # vector -> PE refresh request
        pe_done = es.enter_context(nc.semaphore("pe_done"))  # PE -> vector
        out_sem = es.enter_context(nc.semaphore("out_sem"))
        blk = es.enter_context(nc.Block())

        hist8 = hist[:].bitcast(mybir.dt.float8e4)   # fp8 view (same bytes)

        def tile(t):
            return hist[:, t * FD:(t + 1) * FD]

        def tile8(t):
            return hist8[:, t * FD:(t + 1) * FD]

        @blk.sync
        def _(sync):
            # input rows -> fin[128, 32]: partition 64r+q <- row r cells [32q, 32q+32)
            xr = x[:].rearrange("r (q c) -> (r q) c", c=IE)
            sync.dma_start(fin[:], xr).then_inc(in_sem, 16)
            sync.dma_start(pmat[:, 0:P], pdown[:]).then_inc(in_sem, 16)
            sync.dma_start(pmat[:, P:2 * P], pup[:]).then_inc(in_sem, 16)
            sync.wait_ge(v_sem, 1)
            sync.dma_start(y[:], acc[:]).then_inc(out_sem, 16)
            sync.wait_ge(out_sem, 16)

        @blk.tensor
        def _(tensor):
            tensor.wait_ge(in_sem, 48)
            for k, t in enumerate(ref_steps):
                tensor.wait_ge(pe_go, k + 1)
                # left ghosts: P_down @ interior tail [IE, IE+G)
                nc.tensor.matmul(psum[:, 0:G], pmat[:, 0:P],
                                 tile8(t)[:, IE:IE + G])
                # right ghosts: P_up @ interior head [G, 2G)
                inst = nc.tensor.matmul(psum[:, G:2 * G], pmat[:, P:2 * P],
                                        tile8(t)[:, G:2 * G])
                inst.then_inc(pe_done, 1)

        @blk.vector
        def _(vector):
            # The per-step XOR writes cols [1, FD-1); cols 0 and FD-1 of every
            # tile are read by the next step's OR but always eroded away.
            # Zero them once so reads are defined (and CoreSim is happy).
            h3 = hist[:].rearrange("p (t f) -> p t f", f=FD)
            nc.vector.memset(h3[:, :, 0:1], 0)
            nc.vector.memset(h3[:, :, FD - 1:FD], 0)
            vector.wait_ge(in_sem, 48)
            inst = nc.vector.tensor_scalar(tile8(0)[:, G:G + IE], fin[:],
                                           0.5, None, alu.is_ge)
            inst.then_inc(pe_go, 1)   # tile 0 interior complete -> refresh k=0
            for t in range(T_steps):
                if t in ref_steps:
                    k = ref_steps.index(t)
                    vector.wait_ge(pe_done, k + 1)
                    # Two copies (left/right ghost segments). NOTE: merging
                    # them into one 2-segment strided copy from PSUM passes
                    # CoreSim but corrupts ghost bytes on real hardware --
                    # keep the simple per-segment copies.
                    nc.vector.tensor_copy(tile8(t)[:, 0:G], psum[:, 0:G])
                    nc.vector.tensor_copy(tile8(t)[:, G + IE:FD],
                                          psum[:, G:2 * G])
                s = tile(t)
                d = tile(t + 1)
                # NOTE: erosion-aware shrunken per-step bounds (ops covering
                # only the still-valid [i, FD-i) range) pass analysis but
                # corrupt data on real hardware from mid-window steps onward;
                # keep the fixed full-width ops, which are HW-verified exact.
                nc.vector.tensor_tensor(u[:, 0:FD - 1], s[:, 0:FD - 1], s[:, 1:FD],
                                        alu.bitwise_or)
                inst = nc.vector.tensor_tensor(d[:, 1:FD - 1], u[:, 0:FD - 2],
                                               s[:, 2:FD], alu.bitwise_xor)
                if (t + 1) in ref_steps:
                    inst.then_inc(pe_go, 1)

            # Bit-pack the interiors: byte j of tile t on partition p holds
            # cells [8j, 8j+8) of that partition's 32-cell interior, bit i =
            # cell 8j+i. Interior cells sit at tile cols [G, G+IE) with G a
            # multiple of 8, so col = 8*(G//8 + j) + i.
            hv = hist8.rearrange("p (t j i) -> p t j i", j=FD // 8, i=8)
            av = acc[:].rearrange("p (t j) -> p t j", j=PB)
            tv = tmp[:].rearrange("p (t j) -> p t j", j=PB)
            j0 = G // 8
            nc.vector.tensor_scalar(av, hv[:, :, j0:j0 + PB, 0],
                                    0.5, None, alu.is_ge)
            for i in range(1, 8):
                nc.vector.tensor_scalar(tv, hv[:, :, j0:j0 + PB, i],
                                        0.5, float(1 << i), alu.is_ge, alu.mult)
                inst = nc.vector.tensor_tensor(av, av, tv, alu.add)
            inst.then_inc(v_sem, 1)

    return nc


def _perm_mats():
    """Block-ring permutation matrices (ring within each row's 64 partitions)."""
    import concourse.mybir as mybir
    f8 = mybir.dt.np(mybir.dt.float8e4)
    md = np.zeros((P, P), dtype=np.float32)
    mu = np.zeros((P, P), dtype=np.float32)
    for r in range(2):
        base = r * NR
        q = np.arange(NR)
        md[base + (q - 1) % NR, base + q] = 1.0   # out[m] = in[prev(m)]
        mu[base + (q + 1) % NR, base + q] = 1.0   # out[m] = in[next(m)]
    return md.astype(f8), mu.astype(f8)


def _get_nc(T_steps=T, G=16):
    key = (T_steps, G)
    if key not in _NC_CACHE:
        _NC_CACHE[key] = _build(T_steps, G)
    return _NC_CACHE[key]


def _get_engine():
    """Build (once) the cached sharded-jit dispatch for the 8-core kernel."""
    global _ENGINE
    if _ENGINE is not None:
        return _ENGINE
    import jax
    import concourse.mybir as mybir
    from concourse import bass2jax
    from jax.sharding import Mesh, PartitionSpec, NamedSharding
    from jax.experimental.shard_map import shard_map

    nc = _get_nc()
    bass2jax.install_neuronx_cc_hook()

    partition_name = nc.partition_id_tensor.name if nc.partition_id_tensor else None
    in_names, out_names, out_avals = [], [], []
    for alloc in nc.m.functions[0].allocations:
        if not isinstance(alloc, mybir.MemoryLocationSet):
            continue
        if alloc.kind not in ("ExternalInput", "ExternalOutput"):
            continue
        name = alloc.memorylocations[0].name
        if alloc.kind == "ExternalInput":
            if name != partition_name:
                in_names.append(name)
        else:
            out_names.append(name)
            out_avals.append(jax.core.ShapedArray(
                tuple(alloc.tensor_shape), mybir.dt.np(alloc.dtype)))
    assert in_names == ["x", "pdown", "pup"] and out_names == ["y"], \
        (in_names, out_names)
    all_names = in_names + out_names
    if partition_name is not None:
        all_names = all_names + [partition_name]
    n_params = len(in_names)
    donate = tuple(range(n_params, n_params + len(out_names)))

    def _body(*args):
        operands = list(args)
        if partition_name is not None:
            operands.append(bass2jax.partition_id_tensor())
        outs = bass2jax._bass_exec_p.bind(
            *operands,
            out_avals=tuple(out_avals),
            in_names=tuple(all_names),
            out_names=tuple(out_names),
            lowering_input_output_aliases=(),
            sim_require_finite=True,
            sim_require_nnan=True,
            nc=nc,
        )
        return tuple(outs)

    devices = jax.devices()[:NCORES]
    mesh = Mesh(np.asarray(devices), ("core",))
    nspecs = n_params + len(out_names)
    fn = jax.jit(
        shard_map(_body, mesh=mesh,
                  in_specs=(PartitionSpec("core"),) * nspecs,
                  out_specs=(PartitionSpec("core"),) * len(out_names),
                  check_rep=False),
        donate_argnums=donate, keep_unused=True,
    )

    md, mu = _perm_mats()
    sh = NamedSharding(mesh, PartitionSpec("core"))
    pd = jax.device_put(np.concatenate([md] * NCORES, axis=0), sh)
    pu = jax.device_put(np.concatenate([mu] * NCORES, axis=0), sh)
    pd.block_until_ready(), pu.block_until_ready()

    _ENGINE = {"fn": fn, "pd": pd, "pu": pu, "prev": None, "sh": sh, "pool":
               ThreadPoolExecutor(max_workers=NCORES)}
    return _ENGINE


def _run_fast(inp, out):
    """inp: [16, 2048] f32; unpacks the packed history into out [16, NT, W]."""
    import jax
    eng = _get_engine()
    x = np.ascontiguousarray(inp, dtype=np.float32)
    yinit = eng["prev"]
    eng["prev"] = None
    if yinit is None:
        # device-side init buffer so every call shares one jit signature
        # (its contents are irrelevant -- the kernel writes every byte)
        yinit = jax.device_put(np.zeros((NCORES * P, NT * PB), np.uint8),
                               eng["sh"])
    outs = eng["fn"](x, eng["pd"], eng["pu"], yinit)
    yg = outs[0]

    def fetch(s):
        i = s.index[0].start // P
        a = np.asarray(s.data)                              # [P, NT*PB]
        a = a.reshape(2, NR, NT, PB).transpose(0, 2, 1, 3)  # row, t, q, j
        a = np.ascontiguousarray(a).reshape(2, NT, NR * PB)
        out[2 * i:2 * i + 2] = np.unpackbits(a, axis=-1, bitorder="little")
    list(eng["pool"].map(fetch, yg.addressable_shards))
    eng["prev"] = yg
    return out


def _run_spmd(inp):
    """Fallback: same kernel through run_bass_kernel_spmd (slower per call)."""
    from concourse.bass_utils import run_bass_kernel_spmd
    nc = _get_nc()
    md, mu = _perm_mats()
    in_maps = [{"x": np.ascontiguousarray(inp[2 * i:2 * i + 2], dtype=np.float32),
                "pdown": md, "pup": mu} for i in range(NCORES)]
    res = run_bass_kernel_spmd(nc, in_maps, core_ids=list(range(NCORES)))
    return np.stack([res.results[i]["y"] for i in range(NCORES)])


def _unpack(y_all):
    """[NCORES, P, NT*PB] packed -> [16, NT, 2048] uint8 (0/1)."""
    a = y_all.reshape(NCORES, 2, NR, NT, PB)          # core, row, q, t, j
    a = np.ascontiguousarray(a.transpose(0, 1, 3, 2, 4))  # core, row, t, q, j
    a = a.reshape(B, NT, NR * PB)
    return np.unpackbits(a, axis=-1, bitorder="little")


def run_ca(inp):
    """inp: [16, 2048] f32. Returns [16, T+1, 2048] uint8 via the fast path,
    falling back to run_bass_kernel_spmd dispatch on any failure."""
    global _ENGINE
    out = np.empty((B, NT, W), np.uint8)
    try:
        return _run_fast(inp, out)
    except Exception:
        _ENGINE = None
        return _unpack(_run_spmd(inp))


def _ca_reference_np(inp, lookup, iters):
    s = (inp >= 0.5).astype(np.uint8)
    hist = [s]
    for _ in range(iters):
        pad = np.concatenate([s[:, -1:], s, s[:, :1]], axis=1)
        idx = pad[:, :-2].astype(np.int32) + 2 * pad[:, 1:-1] + 4 * pad[:, 2:]
        s = lookup[idx].astype(np.uint8)
        hist.append(s)
    return np.stack(hist, axis=1)


def kernel(**inputs):
    inp = np.asarray(inputs["input"], dtype=np.float32)
    lookup = np.asarray(inputs["lookup"], dtype=np.uint8)
    if inp.shape != (B, W) or not np.array_equal(lookup, RULE_TABLE):
        # generic (non-rule-30 / odd-shape) fallback
        return _ca_reference_np(inp, lookup, T)
    return run_ca(inp)
